# revision 1
# baseline (speedup 1.0000x reference)
"""Local (sliding-window) self-attention Bass kernel for 8 TRN2 NeuronCores.

Problem: B=4, T=4096, C=512, H=8 heads, head_dim=64, window=15.
Sharding: 8 cores = batch(4) x seq-halves(2). Each core processes 2048 query
tokens of one batch element; its x chunk carries a 7-token halo on each side
(zero-padded at sequence edges, matching the reference's jnp.pad semantics),
padded to 2080 rows for 128/32 alignment.

Per-core dataflow (bf16 matmuls, fp32 PSUM accumulation):
  x chunk --mask*cast--> x_tok bf16 --XBAR DMA transpose--> xT (feature-major)
  qT = Wq-stationary matmuls + bias (feature-major, scores lhsT)
  kT = Wkv[:, :C]-stationary matmuls + bias (feature-major, scores rhs)
  v_tok = xT-stationary matmuls + bias (token-major, AV rhs-source)
  per 128-token block x head-pair:
    scores [128q, 160k] matmul -> exp(scale*s) on ACT -> band-mask*accum on DVE
    -> normalize rows -> XBAR DMA transpose -> alphaT
    attnT [2*64d, 128q] = v.T @ alphaT (two matmuls, K=128 + K=32)
  out = attnT-stationary proj matmuls + bproj + mask -> DMA (token-major)
"""

import math
import os
from contextlib import ExitStack

import ml_dtypes
import numpy as np

import concourse.bacc as bacc
import concourse.bass as bass
import concourse.mybir as mybir
import concourse.tile as tile
from concourse import bass_utils

B, T, C, H, WIN = 4, 4096, 512, 8, 15
D = C // H            # 64
PAD = WIN // 2        # 7
NTOK = T // 2         # 2048 query tokens per core
NKV = 2080            # kv rows per core: 7 + 2048 + 7 = 2062, padded to 2080
NB = NTOK // 128      # 16 query blocks
KCH = [512, 512, 512, 512, 32]  # kv token chunks for feature-major matmuls
SCALE = math.log(WIN) / D
F32 = mybir.dt.float32
BF16 = mybir.dt.bfloat16


def _band_mask() -> np.ndarray:
    """[128,160] multiplicative band: band[p, j] = 1 iff p <= j <= p+14."""
    p = np.arange(128)[:, None]
    j = np.arange(160)[None, :]
    return ((j >= p) & (j <= p + WIN - 1)).astype(ml_dtypes.bfloat16)


def build_program() -> bacc.Bacc:
    nc = bacc.Bacc("TRN2", target_bir_lowering=False, debug=False,
                   enable_asserts=False, num_devices=8)

    xd = nc.dram_tensor("x", [NKV, C], F32, kind="ExternalInput").ap()
    maskd = nc.dram_tensor("mask", [NKV], F32, kind="ExternalInput").ap()
    wqd = nc.dram_tensor("wq", [C, C], F32, kind="ExternalInput").ap()
    bqd = nc.dram_tensor("bq", [C], F32, kind="ExternalInput").ap()
    wkvd = nc.dram_tensor("wkv", [C, 2 * C], F32, kind="ExternalInput").ap()
    bkvd = nc.dram_tensor("bkv", [2 * C], F32, kind="ExternalInput").ap()
    wpd = nc.dram_tensor("wproj", [C, C], F32, kind="ExternalInput").ap()
    bpd = nc.dram_tensor("bproj", [C], F32, kind="ExternalInput").ap()
    bandd = nc.dram_tensor("band", [128, 160], BF16, kind="ExternalInput").ap()
    outd = nc.dram_tensor("out", [NTOK, C], F32, kind="ExternalOutput").ap()

    with tile.TileContext(nc) as tc, ExitStack() as ctx:
        sb = ctx.enter_context(tc.tile_pool(name="sb", bufs=1))
        sb_x = ctx.enter_context(tc.tile_pool(name="sb_x", bufs=3))
        sb_a = ctx.enter_context(tc.tile_pool(name="sb_a", bufs=3))
        sb_o = ctx.enter_context(tc.tile_pool(name="sb_o", bufs=3))
        pp_big = ctx.enter_context(tc.tile_pool(name="pp_big", bufs=2, space="PSUM"))
        pp_sc = ctx.enter_context(tc.tile_pool(name="pp_sc", bufs=4, space="PSUM"))
        pp_at = ctx.enter_context(tc.tile_pool(name="pp_at", bufs=2, space="PSUM"))

        # ---- persistent SBUF tensors ----
        xT = [sb.tile([128, NKV], BF16, tag=f"xT{i}", name=f"xT{i}") for i in range(4)]
        qT = [sb.tile([128, NTOK], BF16, tag=f"qT{i}", name=f"qT{i}") for i in range(4)]
        kT = [sb.tile([128, NKV], BF16, tag=f"kT{i}", name=f"kT{i}") for i in range(4)]
        v_tok = [sb.tile([128, C], BF16, tag=f"vtok{i}", name=f"vtok{i}") for i in range(17)]
        aT = [sb.tile([128, NTOK], BF16, tag=f"aT{i}", name=f"aTt{i}") for i in range(4)]
        band = sb.tile([128, 160], BF16, tag="band")
        wq = [sb.tile([128, C], BF16, tag=f"wq{i}", name=f"wq{i}") for i in range(4)]
        wk = [sb.tile([128, C], BF16, tag=f"wk{i}", name=f"wk{i}") for i in range(4)]
        wv = [sb.tile([128, C], BF16, tag=f"wv{i}", name=f"wv{i}") for i in range(4)]
        wp = [sb.tile([128, C], BF16, tag=f"wp{i}", name=f"wp{i}") for i in range(4)]
        bq_t = sb.tile([128, 4], F32, tag="bq")       # per-partition q bias
        bk_t = sb.tile([128, 4], F32, tag="bk")       # per-partition k bias
        bvB = sb.tile([128, C], F32, tag="bvB")       # v bias bcast over partitions
        bpB = sb.tile([128, C], F32, tag="bpB")       # proj bias bcast
        mq = sb.tile([128, NB], F32, tag="mq")        # query-token mask, per block
        al_bufs = [sb.tile([128, 448], BF16, tag=f"al{j}", name=f"al{j}")
                   for j in range(3)]
        for j in range(3):
            nc.gpsimd.memset(al_bufs[j][:, 320:448], 0.0)

        # ---- constants / weights in ----
        _REP = int(os.environ.get("KREP", "1"))
        for _rep in range(_REP):
         nc.sync.dma_start(band[:], bandd)
         nc.sync.dma_start(bq_t[:], bqd.rearrange("(a b) -> b a", b=128))
         nc.sync.dma_start(bk_t[:], bkvd[0:C].rearrange("(a b) -> b a", b=128))
         nc.sync.dma_start(bvB[:], bkvd[C:2 * C][None, :].broadcast_to((128, C)))
         nc.sync.dma_start(bpB[:], bpd[None, :].broadcast_to((128, C)))
         nc.sync.dma_start(mq[:], maskd[PAD:PAD + NTOK].rearrange("(a b) -> b a", b=128))
         for ci in range(4):
             wqf = sb_x.tile([128, C], F32, tag="wld")
             nc.sync.dma_start(wqf[:], wqd[ci * 128:(ci + 1) * 128, :])
             nc.vector.tensor_copy(wq[ci][:], wqf[:])
             wkf = sb_x.tile([128, 2 * C], F32, tag="wld2")
             nc.sync.dma_start(wkf[:], wkvd[ci * 128:(ci + 1) * 128, :])
             nc.vector.tensor_copy(wk[ci][:], wkf[:, 0:C])
             nc.vector.tensor_copy(wv[ci][:], wkf[:, C:2 * C])
             wpf = sb_x.tile([128, C], F32, tag="wld")
             nc.sync.dma_start(wpf[:], wpd[ci * 128:(ci + 1) * 128, :])
             nc.vector.tensor_copy(wp[ci][:], wpf[:])

         # ---- x in: mask*cast, then XBAR-transpose to feature-major ----
         for t in range(17):
             r0, r1 = t * 128, min((t + 1) * 128, NKV)
             rows = r1 - r0
             xf = sb_x.tile([128, C], F32, tag="xf")
             nc.sync.dma_start(xf[:rows, :], xd[r0:r1, :])
             mrow = sb_x.tile([128, 1], F32, tag="mrow")
             nc.sync.dma_start(mrow[:rows, :], maskd[r0:r1][:, None])
             xb = sb_x.tile([128, C], BF16, tag="xb")
             nc.vector.tensor_scalar_mul(xb[:rows, :], xf[:rows, :], mrow[:rows, :])
             for ci in range(4):
                 nc.scalar.dma_start_transpose(
                     xT[ci][:, r0:r1], xb[:rows, ci * 128:(ci + 1) * 128])

         # ---- qT (feature-major): W stationary, xT moving ----
         for co in range(4):
             for ch in range(4):
                 t0 = ch * 512
                 ps = pp_big.tile([128, 512], F32, tag="big")
                 for ci in range(4):
                     nc.tensor.matmul(
                         ps[:], wq[ci][:, co * 128:(co + 1) * 128],
                         xT[ci][:, PAD + t0:PAD + t0 + 512],
                         start=(ci == 0), stop=(ci == 3))
                 nc.scalar.activation(qT[co][:, t0:t0 + 512], ps[:],
                                      mybir.ActivationFunctionType.Identity,
                                      bias=bq_t[:, co:co + 1])

         # ---- kT (feature-major) ----
         for co in range(4):
             t0 = 0
             for w in KCH:
                 ps = pp_big.tile([128, 512], F32, tag="big")
                 for ci in range(4):
                     nc.tensor.matmul(
                         ps[:, 0:w], wk[ci][:, co * 128:(co + 1) * 128],
                         xT[ci][:, t0:t0 + w],
                         start=(ci == 0), stop=(ci == 3))
                 nc.scalar.activation(kT[co][:, t0:t0 + w], ps[:, 0:w],
                                      mybir.ActivationFunctionType.Identity,
                                      bias=bk_t[:, co:co + 1])
                 t0 += w

         # ---- v_tok (token-major): xT stationary, Wv moving ----
         for t in range(17):
             r0, r1 = t * 128, min((t + 1) * 128, NKV)
             rows = r1 - r0
             ps = pp_big.tile([128, 512], F32, tag="big")
             for ci in range(4):
                 nc.tensor.matmul(
                     ps[:rows, :], xT[ci][:, r0:r1],
                     wv[ci][:], start=(ci == 0), stop=(ci == 3))
             nc.vector.scalar_tensor_tensor(
                 v_tok[t][:rows, :], ps[:rows, :], 1.0, bvB[:rows, :],
                 op0=mybir.AluOpType.mult, op1=mybir.AluOpType.add)

         # ---- attention: per 128-token block, heads in pairs ----
         _PH = int(os.environ.get("KPHASE", "3"))
         if _PH < 2:
             dbg = sb.tile([128, C], F32, tag="dbg")
             nc.vector.tensor_copy(dbg[:], v_tok[0][:])
             nc.sync.dma_start(outd[0:128, :], dbg[:])
         _KS = int(os.environ.get("KSUB", "4"))
         for i in range(min(NB, int(os.environ.get("KNB", str(NB)))) if _PH >= 2 else 0):
             for hp in range(4):                      # head pair -> c-tile hp
                 al = al_bufs[(i * 4 + hp) % 3]
                 for hh in range(2):                  # head h = 2*hp + hh
                     sc = pp_sc.tile([128, 160], F32, tag="sc")
                     nc.tensor.matmul(
                         sc[:],
                         qT[hp][hh * 64:(hh + 1) * 64, i * 128:(i + 1) * 128],
                         kT[hp][hh * 64:(hh + 1) * 64, i * 128:i * 128 + 160],
                         start=True, stop=True)
                     nc.scalar.activation(al[:, hh * 160:hh * 160 + 160], sc[:],
                                          mybir.ActivationFunctionType.Exp,
                                          scale=SCALE)
                 at_ps = pp_at.tile([128, 128], F32, tag="at")
                 for hh in range(2):
                     a = al[:, hh * 160:hh * 160 + 160]
                     if _KS >= 2:
                         den = sb_a.tile([128, 1], F32, tag="den")
                         nc.vector.scalar_tensor_tensor(
                             a, a, 1.0, band[:],
                             op0=mybir.AluOpType.mult, op1=mybir.AluOpType.mult,
                             accum_out=den[:])
                         rden = sb_a.tile([128, 1], F32, tag="rden")
                         nc.vector.reciprocal(rden[:], den[:])
                         nc.vector.tensor_scalar_mul(a, a, rden[:])
                     if _KS >= 3:
                         aT1 = sb_a.tile([128, 128], BF16, tag="aT1")
                         aT2 = sb_a.tile([128, 128], BF16, tag="aT2")
                         c0 = hh * 160
                         nc.scalar.dma_start_transpose(aT1[:], al[:, c0:c0 + 128])
                         nc.scalar.dma_start_transpose(aT2[:], al[:, c0 + 128:c0 + 256])
                     if _KS >= 4:
                         nc.tensor.matmul(
                             at_ps[hh * 64:(hh + 1) * 64, :],
                             v_tok[i][:, hp * 128 + hh * 64:hp * 128 + (hh + 1) * 64],
                             aT1[:], start=True, stop=False)
                         nc.tensor.matmul(
                             at_ps[hh * 64:(hh + 1) * 64, :],
                             v_tok[i + 1][0:32, hp * 128 + hh * 64:hp * 128 + (hh + 1) * 64],
                             aT2[0:32, :], start=False, stop=True)
                 if _KS >= 4:
                     nc.scalar.activation(aT[hp][:, i * 128:(i + 1) * 128], at_ps[:],
                                          mybir.ActivationFunctionType.Copy)

         # ---- proj (token-major): attnT stationary, Wproj moving ----
         if _PH == 2:
             dbg = sb.tile([128, 320], F32, tag="dbg")
             nc.vector.tensor_copy(dbg[:], aT[0][:, 0:320] if _KS >= 4 else al_bufs[0][:, 0:320])
             nc.sync.dma_start(outd[0:128, 0:320], dbg[:])
         for t in range(NB if _PH >= 3 else 0):
             ps = pp_big.tile([128, 512], F32, tag="big")
             for ci in range(4):
                 nc.tensor.matmul(
                     ps[:], aT[ci][:, t * 128:(t + 1) * 128],
                     wp[ci][:], start=(ci == 0), stop=(ci == 3))
             bm = sb_o.tile([128, C], F32, tag="bm")
             nc.gpsimd.tensor_scalar_mul(bm[:], bpB[:], mq[:, t:t + 1])
             ot = sb_o.tile([128, C], F32, tag="ot")
             nc.vector.scalar_tensor_tensor(
                 ot[:], ps[:], mq[:, t:t + 1], bm[:],
                 op0=mybir.AluOpType.mult, op1=mybir.AluOpType.add)
             nc.sync.dma_start(outd[t * 128:(t + 1) * 128, :], ot[:])

    nc.compile()
    return nc


_CACHE: dict = {}


def _get_program() -> bacc.Bacc:
    if "nc" not in _CACHE:
        _CACHE["nc"] = build_program()
    return _CACHE["nc"]


def kernel(x, mask, Wq, bq, Wkv, bkv, Wproj, bproj) -> np.ndarray:
    x = np.asarray(x, np.float32)
    mask = np.asarray(mask, np.float32)
    band = np.ascontiguousarray(_band_mask())
    nc = _get_program()

    in_maps = []
    for core in range(8):
        b, h = divmod(core, 2)
        s = h * NTOK
        xc = np.zeros((NKV, C), np.float32)
        mc = np.zeros((NKV,), np.float32)
        lo, hi = max(0, s - PAD), min(T, s + NTOK + PAD)
        xc[lo - (s - PAD):lo - (s - PAD) + hi - lo] = x[b, lo:hi]
        mc[lo - (s - PAD):lo - (s - PAD) + hi - lo] = mask[b, lo:hi]
        in_maps.append({
            "x": xc, "mask": mc,
            "wq": np.asarray(Wq, np.float32), "bq": np.asarray(bq, np.float32),
            "wkv": np.asarray(Wkv, np.float32), "bkv": np.asarray(bkv, np.float32),
            "wproj": np.asarray(Wproj, np.float32),
            "bproj": np.asarray(bproj, np.float32),
            "band": band,
        })

    res = bass_utils.run_bass_kernel_spmd(nc, in_maps, core_ids=list(range(8)))
    out = np.empty((B, T, C), np.float32)
    for core in range(8):
        b, h = divmod(core, 2)
        out[b, h * NTOK:(h + 1) * NTOK] = res.results[core]["out"]
    return out



# revision 19
# speedup vs baseline: 4.6935x; 4.6935x over previous
"""Local (sliding-window) self-attention Bass kernel for 8 TRN2 NeuronCores.

Problem: B=4, T=4096, C=512, H=8 heads, head_dim=64, window=15.
Sharding: 8 cores = batch(4) x seq-halves(2). Each core processes 2048 query
tokens of one batch element; its x chunk carries a 7-token halo on each side
(zero-padded at sequence edges, matching the reference's jnp.pad semantics),
padded to 2080 rows.

Per-core dataflow (bf16 matmuls, fp32 PSUM accumulation):
  x bf16 [NKV,C] --PE matmul vs diag(mask)--> xT (feature-major, mask fused)
  qT/kT feature-major: W-stationary matmuls + bias on ACT
  v per 114-token block: x-stationary matmul, bias on DVE, stored head-major
    with a ones column per head (65-stride) for the softmax denominator
  attention per 114-token block (kv extent 128, no K-split):
    scores kv-major [128kv, W]: 1 matmul/head -> exp on ACT -> band mask DVE
    AV token-major [W, 65]/head: lhsT=alphaT, rhs=v|1 -> denom in col 64
    reciprocal + per-partition normalize on DVE -> attn_tok bf16
    PE is_transpose -> attnT feature-major
  proj: attnT-stationary matmuls; bias add + mask on DVE; bf16 out
"""

import math
from contextlib import ExitStack

import ml_dtypes
import numpy as np

import concourse.bacc as bacc
import concourse.bass as bass
import concourse.mybir as mybir
import concourse.tile as tile
from concourse import bass_utils

B, T, C, H, WIN = 4, 4096, 512, 8, 15
D = C // H            # 64
PAD = WIN // 2        # 7
NTOK = T // 2         # 2048 query tokens per core
NKV = 2080            # kv rows per core: 7 + 2048 + 7 = 2062, padded to 2080
QB = 114              # query block (kv extent = QB + WIN - 1 = 128)
NQB = 18              # 17 full blocks + one 110-wide block
KCH = [512, 512, 512, 512, 32]  # kv chunks for the k GEMM
SCALE = math.log(WIN) / D
F32 = mybir.dt.float32
BF16 = mybir.dt.bfloat16


def _band4() -> np.ndarray:
    """[128, 4*114] bf16: band[p, h*114+j] = 1 iff 0 <= p - j <= 14."""
    p = np.arange(128)[:, None]
    j = np.arange(QB)[None, :]
    band = ((p >= j) & (p <= j + WIN - 1)).astype(ml_dtypes.bfloat16)
    return np.ascontiguousarray(np.tile(band, (1, 4)))


def _blk_w(b: int) -> int:
    return min(QB, NTOK - b * QB)


def build_program() -> bacc.Bacc:
    nc = bacc.Bacc("TRN2", target_bir_lowering=False, debug=False,
                   enable_asserts=False, num_devices=8)

    xd = nc.dram_tensor("x", [NKV, C], BF16, kind="ExternalInput").ap()
    maskd = nc.dram_tensor("mask", [17 * 128], F32, kind="ExternalInput").ap()
    wqd = nc.dram_tensor("wq", [C, C], BF16, kind="ExternalInput").ap()
    wkd = nc.dram_tensor("wk", [C, C], BF16, kind="ExternalInput").ap()
    wvd = nc.dram_tensor("wv", [C, C], BF16, kind="ExternalInput").ap()
    wpd = nc.dram_tensor("wp", [C, C], BF16, kind="ExternalInput").ap()
    bqd = nc.dram_tensor("bq", [C], F32, kind="ExternalInput").ap()
    bkd = nc.dram_tensor("bk", [C], F32, kind="ExternalInput").ap()
    bvd = nc.dram_tensor("bv", [C], F32, kind="ExternalInput").ap()
    bpd = nc.dram_tensor("bp", [C], F32, kind="ExternalInput").ap()
    bandd = nc.dram_tensor("band4", [128, 4 * QB], BF16, kind="ExternalInput").ap()
    eyed = nc.dram_tensor("eye", [128, 128], BF16, kind="ExternalInput").ap()
    outd = nc.dram_tensor("out", [NTOK, C], BF16, kind="ExternalOutput").ap()

    with tile.TileContext(nc) as tc, ExitStack() as ctx:
        sb = ctx.enter_context(tc.tile_pool(name="sb", bufs=1))
        sb_x = ctx.enter_context(tc.tile_pool(name="sb_x", bufs=4))
        sb_e = ctx.enter_context(tc.tile_pool(name="sb_e", bufs=4))
        sb_al = ctx.enter_context(tc.tile_pool(name="sb_al", bufs=4))
        sb_at = ctx.enter_context(tc.tile_pool(name="sb_at", bufs=3))
        sb_rd = ctx.enter_context(tc.tile_pool(name="sb_rd", bufs=4))
        sb_o = ctx.enter_context(tc.tile_pool(name="sb_o", bufs=3))
        pp_big = ctx.enter_context(tc.tile_pool(name="pp_big", bufs=2, space="PSUM"))
        pp_sc = ctx.enter_context(tc.tile_pool(name="pp_sc", bufs=2, space="PSUM"))
        pp_at = ctx.enter_context(tc.tile_pool(name="pp_at", bufs=2, space="PSUM"))
        pp_tr = ctx.enter_context(tc.tile_pool(name="pp_tr", bufs=2, space="PSUM"))

        # ---- persistent SBUF ----
        xT = sb.tile([128, 4 * NKV], BF16, tag="xT")     # feature-major masked x
        qT = sb.tile([128, 8 * NTOK], BF16, tag="qT")  # per-head, zero-padded halves
        kT = sb.tile([128, 4 * NKV], BF16, tag="kT")
        aT = sb.tile([128, 4 * NTOK], BF16, tag="aT")    # feature-major attn
        vb = [sb.tile([128, 8 * 65], BF16, tag=f"vb{b}", name=f"vb{b}")
              for b in range(NQB)]
        wq_t = sb.tile([128, 4 * C], BF16, tag="wq")
        wk_t = sb.tile([128, 4 * C], BF16, tag="wk")
        wv_t = sb.tile([128, 4 * C], BF16, tag="wv")
        wp_t = sb.tile([128, 4 * C], BF16, tag="wp")
        bq_t = sb.tile([128, 4], F32, tag="bq")
        bk_t = sb.tile([128, 4], F32, tag="bk")
        bvB = sb.tile([128, C], F32, tag="bvB")
        bpB = sb.tile([128, C], F32, tag="bpB")
        mcol = sb.tile([128, 17], F32, tag="mcol")       # mask per x tile
        mq = sb.tile([128, 16], F32, tag="mq")           # query mask per 128-blk
        band = sb.tile([128, 4 * QB], BF16, tag="band")
        eye = sb.tile([128, 128], BF16, tag="eye")

        # ---- prologue DMAs (sync queue: consts + weights; scalar queue: x) ----
        nc.sync.dma_start(band[:], bandd)
        nc.sync.dma_start(eye[:], eyed)
        nc.sync.dma_start(mcol[:], maskd.rearrange("(a b) -> b a", b=128))
        nc.sync.dma_start(mq[:], maskd[PAD:PAD + NTOK].rearrange("(a b) -> b a", b=128))
        nc.sync.dma_start(bq_t[:], bqd.rearrange("(a b) -> b a", b=128))
        nc.sync.dma_start(bk_t[:], bkd.rearrange("(a b) -> b a", b=128))
        nc.sync.dma_start(bvB[:], bvd[None, :].broadcast_to((128, C)))
        nc.sync.dma_start(bpB[:], bpd[None, :].broadcast_to((128, C)))
        for ci in range(4):
            nc.sync.dma_start(wq_t[:, ci * C:(ci + 1) * C], wqd[ci * 128:(ci + 1) * 128, :])
            nc.sync.dma_start(wk_t[:, ci * C:(ci + 1) * C], wkd[ci * 128:(ci + 1) * 128, :])
            nc.sync.dma_start(wv_t[:, ci * C:(ci + 1) * C], wvd[ci * 128:(ci + 1) * 128, :])
            nc.sync.dma_start(wp_t[:, ci * C:(ci + 1) * C], wpd[ci * 128:(ci + 1) * 128, :])
        # ones columns (softmax denominator) in every v tile
        for b in range(NQB):
            nc.gpsimd.memset(vb[b].rearrange("p (h w) -> p h w", w=65)[:, :, 64:65], 1.0)
        # zero the unused partition half of each head's qT section
        for h in range(8):
            p0 = 0 if h % 2 else 64
            nc.gpsimd.memset(qT[p0:p0 + 64, h * NTOK:(h + 1) * NTOK], 0.0)

        # ---- x in + masked transpose (PE matmul vs diag(mask)) ----
        def emit_xtile(t: int):
            r0 = t * 128
            rows = min(128, NKV - r0)
            xf = sb_x.tile([128, C], BF16, tag="xf")
            nc.scalar.dma_start(xf[:rows, :], xd[r0:r0 + rows, :])
            dg = sb_x.tile([128, 128], BF16, tag="dg")
            nc.vector.tensor_scalar_mul(dg[:rows, :rows], eye[:rows, :rows],
                                        mcol[:rows, t:t + 1])
            ps = pp_big.tile([128, 512], F32, tag="big")
            for ci in range(4):
                nc.tensor.matmul(ps[:, ci * 128:ci * 128 + rows],
                                 xf[:rows, ci * 128:(ci + 1) * 128],
                                 dg[:rows, :rows], start=True, stop=True)
            nc.scalar.activation(
                xT.rearrange("p (c n) -> p c n", c=4)[:, :, r0:r0 + rows],
                ps.rearrange("p (c n) -> p c n", c=4)[:, :, 0:rows],
                mybir.ActivationFunctionType.Copy)

        def emit_q(ch: int):
            t0 = ch * 512
            for co in range(4):
                ps = pp_big.tile([128, 512], F32, tag="big")
                for ci in range(4):
                    nc.tensor.matmul(
                        ps[:], wq_t[:, ci * C + co * 128:ci * C + (co + 1) * 128],
                        xT[:, ci * NKV + PAD + t0:ci * NKV + PAD + t0 + 512],
                        start=(ci == 0), stop=(ci == 3))
                for e in range(2):
                    h = 2 * co + e
                    p0 = 64 * e
                    nc.scalar.activation(
                        qT[p0:p0 + 64, h * NTOK + t0:h * NTOK + t0 + 512],
                        ps[p0:p0 + 64, :],
                        mybir.ActivationFunctionType.Identity,
                        bias=bq_t[p0:p0 + 64, co:co + 1])

        def emit_k(ch: int):
            t0 = sum(KCH[:ch])
            w = KCH[ch]
            for co in range(4):
                ps = pp_big.tile([128, 512], F32, tag="big")
                for ci in range(4):
                    nc.tensor.matmul(
                        ps[:, 0:w], wk_t[:, ci * C + co * 128:ci * C + (co + 1) * 128],
                        xT[:, ci * NKV + t0:ci * NKV + t0 + w],
                        start=(ci == 0), stop=(ci == 3))
                nc.scalar.activation(kT[:, co * NKV + t0:co * NKV + t0 + w], ps[:, 0:w],
                                     mybir.ActivationFunctionType.Identity,
                                     bias=bk_t[:, co:co + 1])

        def emit_v(b: int):
            kv0 = b * QB
            ps = pp_big.tile([128, 512], F32, tag="big")
            for ci in range(4):
                nc.tensor.matmul(ps[:], xT[:, ci * NKV + kv0:ci * NKV + kv0 + 128],
                                 wv_t[:, ci * C:(ci + 1) * C],
                                 start=(ci == 0), stop=(ci == 3))
            nc.vector.scalar_tensor_tensor(
                vb[b].rearrange("p (h w) -> p h w", w=65)[:, :, 0:64],
                ps.rearrange("p (h w) -> p h w", w=64), 1.0,
                bvB.rearrange("p (h w) -> p h w", w=64),
                op0=mybir.AluOpType.mult, op1=mybir.AluOpType.add)

        sc_tiles: dict = {}
        al_tiles: dict = {}
        at_tiles: dict = {}
        rd_tiles: dict = {}
        atok_tiles: dict = {}

        def emit_sc(b: int):
            import os
            KSC = int(os.environ.get("KSC", "3"))
            kv0, q0, w = b * QB, b * QB, _blk_w(b)
            sc_tiles[b] = []
            al_tiles[b] = []
            for g in range(2):
                ps = pp_sc.tile([128, 512], F32, tag="sc")
                for pp in range(2):          # head pair co = g*2 + pp
                    co = g * 2 + pp
                    nc.tensor.matmul(
                        ps[:, pp * 256:pp * 256 + 2 * w],
                        kT[:, co * NKV + kv0:co * NKV + kv0 + 128],
                        qT.rearrange("p (h n) -> p h n", h=8)[:, 2 * co:2 * co + 2, q0:q0 + w],
                        start=True, stop=True)
                ex = sb_e.tile([128, 4 * QB], BF16, tag="ex")
                if KSC >= 2:
                    nc.scalar.activation(
                        ex[:, 0:4 * w].rearrange("p (g n) -> p g n", n=2 * w),
                        ps.rearrange("p (g n) -> p g n", g=2)[:, :, 0:2 * w],
                        mybir.ActivationFunctionType.Exp, scale=SCALE)
                al = sb_al.tile([128, 4 * QB], BF16, tag="al")
                if KSC >= 3:
                    nc.vector.tensor_mul(
                        al.rearrange("p (h w) -> p h w", h=4)[:, :, 0:w],
                        ex[:, 0:4 * w].rearrange("p (h w) -> p h w", w=w),
                        band.rearrange("p (h w) -> p h w", h=4)[:, :, 0:w])
                sc_tiles[b].append(ps)
                al_tiles[b].append(al)

        def emit_av(b: int):
            w = _blk_w(b)
            at_tiles[b] = []
            rd = sb_rd.tile([128, 8], F32, tag="rd")
            atok = sb_at.tile([128, 512], BF16, tag="atok")
            for g in range(2):
                al = al_tiles[b][g]
                ps = pp_at.tile([128, 512], F32, tag="at")
                for hh in range(4):
                    h = g * 4 + hh
                    nc.tensor.matmul(
                        ps[0:w, hh * 128:hh * 128 + 65],
                        al[:, hh * QB:hh * QB + w],
                        vb[b][:, h * 65:(h + 1) * 65],
                        start=True, stop=True)
                nc.vector.reciprocal(
                    rd[0:w, g * 4:(g + 1) * 4],
                    ps.rearrange("p (h w) -> p h w", w=128)[0:w, :, 64])
                at_tiles[b].append(ps)
            for h in range(8):
                g, hh = divmod(h, 4)
                nc.vector.tensor_scalar_mul(
                    atok[0:w, h * 64:(h + 1) * 64],
                    at_tiles[b][g][0:w, hh * 128:hh * 128 + 64],
                    rd[0:w, h:h + 1])
            rd_tiles[b] = rd
            atok_tiles[b] = atok
            del al_tiles[b]

        def emit_tr(b: int):
            w = _blk_w(b)
            q0 = b * QB
            atok = atok_tiles.pop(b)
            ps = pp_tr.tile([128, 1024], BF16, tag="tr")
            for ci in range(4):
                nc.tensor.transpose(ps[:, ci * 256:ci * 256 + w],
                                    atok[0:w, ci * 128:(ci + 1) * 128],
                                    eye[0:w, 0:w])
            nc.scalar.activation(
                aT.rearrange("p (c n) -> p c n", c=4)[:, :, q0:q0 + w],
                ps.rearrange("p (c n) -> p c n", c=4)[:, :, 0:w],
                mybir.ActivationFunctionType.Copy)

        def emit_proj(j: int):
            t0 = j * 128
            ps = pp_big.tile([128, 512], F32, tag="big")
            for ci in range(4):
                nc.tensor.matmul(ps[:], aT[:, ci * NTOK + t0:ci * NTOK + t0 + 128],
                                 wp_t[:, ci * C:(ci + 1) * C],
                                 start=(ci == 0), stop=(ci == 3))
            tmp = sb_o.tile([128, C], F32, tag="tmp")
            nc.vector.tensor_add(tmp[:], ps[:], bpB[:])
            ot = sb_o.tile([128, C], BF16, tag="ot")
            nc.vector.tensor_scalar_mul(ot[:], tmp[:], mq[:, j:j + 1])
            nc.sync.dma_start(outd[t0:t0 + 128, :], ot[:])

        # ---- schedule ----
        import os
        PH = int(os.environ.get("KPH", "7"))
        for t in range(17):
            emit_xtile(t)
            if PH >= 2 and t in (4, 8, 12, 16):
                emit_q(t // 4 - 1)
        kv_done = 0
        k_next = 0
        proj_next = 0
        for b in range(NQB + 2):
            if b < NQB and PH >= 2:
                need = min(b * QB + 128, NKV)
                while kv_done < need:
                    emit_k(k_next)
                    kv_done += KCH[k_next]
                    k_next += 1
                if PH >= 3:
                    emit_v(b)
                if PH >= 4:
                    emit_sc(b)
            if 1 <= b <= NQB and PH >= 5:
                emit_av(b - 1)
            if b >= 2 and PH >= 6:
                emit_tr(b - 2)
                done = min((b - 1) * QB, NTOK)
                while proj_next < 16 and (proj_next + 1) * 128 <= done and PH >= 7:
                    emit_proj(proj_next)
                    proj_next += 1
        while proj_next < 16 and PH >= 7:
            emit_proj(proj_next)
            proj_next += 1
        if PH < 7:
            dbg = sb.tile([128, C], BF16, tag="dbg")
            nc.vector.tensor_copy(dbg[:], bpB[:])
            nc.sync.dma_start(outd[0:128, :], dbg[:])

    nc.compile()
    return nc


_CACHE: dict = {}


def _get_program() -> bacc.Bacc:
    if "nc" not in _CACHE:
        _CACHE["nc"] = build_program()
    return _CACHE["nc"]


def _core_inputs(x, mask, Wq, bq, Wkv, bkv, Wproj, bproj, core: int) -> dict:
    b, h = divmod(core, 2)
    s = h * NTOK
    xc = np.zeros((NKV, C), np.float32)
    mc = np.zeros((17 * 128,), np.float32)
    lo, hi = max(0, s - PAD), min(T, s + NTOK + PAD)
    xc[lo - (s - PAD):lo - (s - PAD) + hi - lo] = x[b, lo:hi]
    mc[lo - (s - PAD):lo - (s - PAD) + hi - lo] = mask[b, lo:hi]
    bf = ml_dtypes.bfloat16
    return {
        "x": xc.astype(bf), "mask": mc,
        "wq": np.asarray(Wq, np.float32).astype(bf),
        "wk": np.ascontiguousarray(np.asarray(Wkv, np.float32)[:, :C]).astype(bf),
        "wv": np.ascontiguousarray(np.asarray(Wkv, np.float32)[:, C:]).astype(bf),
        "wp": np.asarray(Wproj, np.float32).astype(bf),
        "bq": np.asarray(bq, np.float32),
        "bk": np.ascontiguousarray(np.asarray(bkv, np.float32)[:C]),
        "bv": np.ascontiguousarray(np.asarray(bkv, np.float32)[C:]),
        "bp": np.asarray(bproj, np.float32),
        "band4": _band4(),
        "eye": np.ascontiguousarray(np.eye(128, dtype=bf)),
    }


def kernel(x, mask, Wq, bq, Wkv, bkv, Wproj, bproj) -> np.ndarray:
    x = np.asarray(x, np.float32)
    mask = np.asarray(mask, np.float32)
    nc = _get_program()
    in_maps = [_core_inputs(x, mask, Wq, bq, Wkv, bkv, Wproj, bproj, core)
               for core in range(8)]
    res = bass_utils.run_bass_kernel_spmd(nc, in_maps, core_ids=list(range(8)))
    out = np.empty((B, T, C), np.float32)
    for core in range(8):
        b, h = divmod(core, 2)
        out[b, h * NTOK:(h + 1) * NTOK] = np.asarray(res.results[core]["out"],
                                                     dtype=np.float32)
    return out
